# revision 4
# baseline (speedup 1.0000x reference)
import numpy as np

# EnhancedMSTSN — data-parallel over batch across 8 NeuronCores.
# Shapes are fixed by the problem: x [8, 32, 256, 3], params as in setup_inputs().
# Strategy (per sharding hint): shard x on the leading batch axis (1 element per
# core), replicate the tiny parameter set and the N x N adjacency mask (computed
# once on host from params['emb'] — it depends only on parameters, not on x).

B, S, N = 8, 32, 256
EPS_LN = 1e-3


def _adjacency_mask(emb: np.ndarray) -> np.ndarray:
    """Replicates AdaptiveAdjacency from the reference in float32 numpy."""
    emb = emb.astype(np.float32)
    nrm = emb * (1.0 / np.sqrt(np.maximum((emb * emb).sum(-1, keepdims=True), 1e-12)))
    adj = (nrm @ nrm.T).astype(np.float32) * (1.0 - np.eye(N, dtype=np.float32))
    # global top-k(2) threshold
    top2 = np.sort(adj, axis=1)[:, -2:]
    thr = top2.min()
    adj = np.where(adj >= thr, adj, 0.0)
    return (adj > 0.5) | np.eye(N, dtype=bool)


def kernel(x: np.ndarray, params: dict) -> np.ndarray:
    import jax
    import jax.numpy as jnp

    x = np.asarray(x, dtype=np.float32)
    p = jax.tree.map(lambda a: np.asarray(a, dtype=np.float32), params)
    mask_np = _adjacency_mask(np.asarray(p['emb']))

    def _ln(h, g, b):
        m = h.mean(-1, keepdims=True)
        v = ((h - m) ** 2).mean(-1, keepdims=True)
        return (h - m) * jax.lax.rsqrt(v + EPS_LN) * g + b

    def _mha(q_in, kv, Wq, bq, Wk, bk, Wv, bv, Wo, bo):
        scale = np.float32(1.0 / np.sqrt(Wq.shape[-1]))
        q = jnp.einsum('bnd,dhc->bnhc', q_in, Wq) + bq
        k = jnp.einsum('bnd,dhc->bnhc', kv, Wk) + bk
        v = jnp.einsum('bnd,dhc->bnhc', kv, Wv) + bv
        sc = jnp.einsum('bqhc,bkhc->bhqk', q, k) * scale
        a = jax.nn.softmax(sc, axis=-1)
        o = jnp.einsum('bhqk,bkhc->bqhc', a, v)
        return jnp.einsum('bqhc,hco->bqo', o, Wo) + bo

    def _gat(h, nbr, valid, gp):
        # Sparse slot formulation: per node i, the ≤D neighbors nbr[i, :].
        # Identical numerics to the masked dense softmax: non-edges get
        # exp(-1e9 - max) == 0 exactly in fp32, so only edges contribute.
        W, a_s, a_d, bb = gp['W'], gp['a_src'], gp['a_dst'], gp['b']
        Bn, Nn, _ = h.shape
        H, C = a_s.shape
        hf = (h @ W).reshape(Bn, Nn, H, C)
        s = (hf * a_s).sum(-1)                       # [Bn, N, H]
        t = (hf * a_d).sum(-1)
        t_sl = t[:, nbr, :]                          # [Bn, N, D, H]
        e = jax.nn.leaky_relu(s[:, :, None, :] + t_sl, negative_slope=0.2)
        w = jnp.exp(e - e.max(axis=2, keepdims=True)) * valid[None, :, :, None]
        z = w.sum(axis=2, keepdims=True)             # [Bn, N, 1, H]
        alpha = w / z
        hf_sl = hf[:, nbr, :, :]                     # [Bn, N, D, H, C]
        out = jnp.einsum('bndh,bndhc->bnhc', alpha, hf_sl).reshape(Bn, Nn, H * C)
        return out + bb

    def fwd(xb, prm, nbr, valid):
        # xb: [1, S, N, 3] — one batch element on this device
        Bq = xb.shape[0]
        xs = xb.reshape(Bq * S, N, 3)
        h = xs @ prm['proj_W'] + prm['proj_b']
        h = jax.nn.relu(_gat(h, nbr, valid, prm['gat1']))
        h = _gat(h, nbr, valid, prm['gat2'])
        spatial_out = h.reshape(Bq, S, N, 32)
        t_in = spatial_out.reshape(Bq * N, S, 32)
        attn = _mha(t_in, t_in, prm['tWq'], prm['tbq'], prm['tWk'], prm['tbk'],
                    prm['tWv'], prm['tbv'], prm['tWo'], prm['tbo'])
        out1 = _ln(t_in + attn, prm['ln1_g'], prm['ln1_b'])
        ffn = jax.nn.gelu(out1 @ prm['fW1'] + prm['fb1'],
                          approximate=False) @ prm['fW2'] + prm['fb2']
        t_out = _ln(out1 + ffn, prm['ln2_g'], prm['ln2_b'])
        temporal_out = t_out.reshape(Bq, N, S, 32)
        spatial_feats = spatial_out.mean(axis=1)
        temporal_feats = temporal_out.mean(axis=2)
        fused = _mha(spatial_feats, temporal_feats,
                     prm['cWq'], prm['cbq'], prm['cWk'], prm['cbk'],
                     prm['cWv'], prm['cbv'], prm['cWo'], prm['cbo'])
        hid = jax.nn.gelu(fused @ prm['rW1'] + prm['rb1'], approximate=False)
        return (hid @ prm['rW2'] + prm['rb2'])[..., 0]

    # neighbor slots from the mask (host): nbr [N, D] int32 (padded with i),
    # valid [N, D] float32
    deg = mask_np.sum(1)
    D = int(deg.max())
    nbr_np = np.tile(np.arange(N, dtype=np.int32)[:, None], (1, D))
    valid_np = np.zeros((N, D), dtype=np.float32)
    for i in range(N):
        js = np.nonzero(mask_np[i])[0]
        nbr_np[i, :len(js)] = js
        valid_np[i, :len(js)] = 1.0

    n_dev = min(8, jax.device_count())
    assert B % n_dev == 0
    xsh = x.reshape(n_dev, B // n_dev, S, N, 3)
    run = jax.pmap(fwd, in_axes=(0, None, None, None))
    out = run(xsh, p, jnp.asarray(nbr_np), jnp.asarray(valid_np))
    return np.asarray(out).reshape(B, N).astype(np.float32)


if __name__ == '__main__':
    import reference
    ins = reference.setup_inputs()
    got = kernel(**{k: np.asarray(v) if not isinstance(v, dict) else v
                    for k, v in ins.items()})
    print(got.shape, got.dtype)


# revision 8
# speedup vs baseline: 1.5702x; 1.5702x over previous
import numpy as np

# EnhancedMSTSN — data-parallel over batch across 8 NeuronCores.
# Shapes are fixed by the problem: x [8, 32, 256, 3], params as in setup_inputs().
# Strategy (per sharding hint): shard x on the leading batch axis (1 element per
# core), replicate the tiny parameter set and the N x N adjacency mask (computed
# once on host from params['emb'] — it depends only on parameters, not on x).

B, S, N = 8, 32, 256
EPS_LN = 1e-3


def _adjacency_mask(emb: np.ndarray) -> np.ndarray:
    """Replicates AdaptiveAdjacency from the reference in float32 numpy."""
    emb = emb.astype(np.float32)
    nrm = emb * (1.0 / np.sqrt(np.maximum((emb * emb).sum(-1, keepdims=True), 1e-12)))
    adj = (nrm @ nrm.T).astype(np.float32) * (1.0 - np.eye(N, dtype=np.float32))
    # global top-k(2) threshold
    top2 = np.sort(adj, axis=1)[:, -2:]
    thr = top2.min()
    adj = np.where(adj >= thr, adj, 0.0)
    return (adj > 0.5) | np.eye(N, dtype=bool)


def kernel(x: np.ndarray, params: dict) -> np.ndarray:
    import jax
    import jax.numpy as jnp

    x = np.asarray(x, dtype=np.float32)
    p = jax.tree.map(lambda a: np.asarray(a, dtype=np.float32), params)
    mask_np = _adjacency_mask(np.asarray(p['emb']))

    def _ln(h, g, b):
        m = h.mean(-1, keepdims=True)
        v = ((h - m) ** 2).mean(-1, keepdims=True)
        return (h - m) * jax.lax.rsqrt(v + EPS_LN) * g + b

    def _mha(q_in, kv, Wq, bq, Wk, bk, Wv, bv, Wo, bo):
        scale = np.float32(1.0 / np.sqrt(Wq.shape[-1]))
        q = jnp.einsum('bnd,dhc->bnhc', q_in, Wq) + bq
        k = jnp.einsum('bnd,dhc->bnhc', kv, Wk) + bk
        v = jnp.einsum('bnd,dhc->bnhc', kv, Wv) + bv
        sc = jnp.einsum('bqhc,bkhc->bhqk', q, k) * scale
        a = jax.nn.softmax(sc, axis=-1)
        o = jnp.einsum('bhqk,bkhc->bqhc', a, v)
        return jnp.einsum('bqhc,hco->bqo', o, Wo) + bo

    def _gat(h, mbias, gp):
        # Dense formulation with additive mask bias and no max-subtraction:
        # e is bounded (|e| < ~60) so exp cannot overflow in fp32, and masked
        # entries see exp(x - 1e9) == 0 exactly — identical numerics to the
        # reference's where(mask, e, -1e9) + softmax.
        W, a_s, a_d, bb = gp['W'], gp['a_src'], gp['a_dst'], gp['b']
        Bn, Nn, _ = h.shape
        H, C = a_s.shape
        hf = (h @ W).reshape(Bn, Nn, H, C)
        s = (hf * a_s).sum(-1)                       # [Bn, N, H]
        t = (hf * a_d).sum(-1)
        e = jax.nn.leaky_relu(
            s[:, :, None, :] + t[:, None, :, :], negative_slope=0.2)
        w = jnp.exp(e + mbias[None, :, :, None])     # [Bn, i, j, H]
        alpha = w / w.sum(axis=2, keepdims=True)
        out = jnp.einsum('bijh,bjhc->bihc', alpha, hf).reshape(Bn, Nn, H * C)
        return out + bb

    def fwd(xb, prm, mbias):
        # xb: [1, S, N, 3] — one batch element on this device
        Bq = xb.shape[0]
        xs = xb.reshape(Bq * S, N, 3)
        h = xs @ prm['proj_W'] + prm['proj_b']
        h = jax.nn.relu(_gat(h, mbias, prm['gat1']))
        h = _gat(h, mbias, prm['gat2'])
        spatial_out = h.reshape(Bq, S, N, 32)
        t_in = spatial_out.reshape(Bq * N, S, 32)
        attn = _mha(t_in, t_in, prm['tWq'], prm['tbq'], prm['tWk'], prm['tbk'],
                    prm['tWv'], prm['tbv'], prm['tWo'], prm['tbo'])
        out1 = _ln(t_in + attn, prm['ln1_g'], prm['ln1_b'])
        ffn = jax.nn.gelu(out1 @ prm['fW1'] + prm['fb1'],
                          approximate=False) @ prm['fW2'] + prm['fb2']
        t_out = _ln(out1 + ffn, prm['ln2_g'], prm['ln2_b'])
        temporal_out = t_out.reshape(Bq, N, S, 32)
        spatial_feats = spatial_out.mean(axis=1)
        temporal_feats = temporal_out.mean(axis=2)
        fused = _mha(spatial_feats, temporal_feats,
                     prm['cWq'], prm['cbq'], prm['cWk'], prm['cbk'],
                     prm['cWv'], prm['cbv'], prm['cWo'], prm['cbo'])
        hid = jax.nn.gelu(fused @ prm['rW1'] + prm['rb1'], approximate=False)
        return (hid @ prm['rW2'] + prm['rb2'])[..., 0]

    mbias_np = np.where(mask_np, np.float32(0.0), np.float32(-1e9))

    n_dev = min(8, jax.device_count())
    assert B % n_dev == 0
    xsh = x.reshape(n_dev, B // n_dev, S, N, 3)
    run = jax.pmap(fwd, in_axes=(0, None, None))
    out = run(xsh, p, jnp.asarray(mbias_np))
    return np.asarray(out).reshape(B, N).astype(np.float32)


if __name__ == '__main__':
    import reference
    ins = reference.setup_inputs()
    got = kernel(**{k: np.asarray(v) if not isinstance(v, dict) else v
                    for k, v in ins.items()})
    print(got.shape, got.dtype)


# revision 10
# speedup vs baseline: 1.8627x; 1.1862x over previous
import numpy as np

# EnhancedMSTSN — data-parallel over batch across 8 NeuronCores.
# Shapes are fixed by the problem: x [8, 32, 256, 3], params as in setup_inputs().
# Strategy (per sharding hint): shard x on the leading batch axis (1 element per
# core), replicate the tiny parameter set and the N x N adjacency mask (computed
# once on host from params['emb'] — it depends only on parameters, not on x).

B, S, N = 8, 32, 256
EPS_LN = 1e-3


def _adjacency_mask(emb: np.ndarray) -> np.ndarray:
    """Replicates AdaptiveAdjacency from the reference in float32 numpy."""
    emb = emb.astype(np.float32)
    nrm = emb * (1.0 / np.sqrt(np.maximum((emb * emb).sum(-1, keepdims=True), 1e-12)))
    adj = (nrm @ nrm.T).astype(np.float32) * (1.0 - np.eye(N, dtype=np.float32))
    # global top-k(2) threshold
    top2 = np.sort(adj, axis=1)[:, -2:]
    thr = top2.min()
    adj = np.where(adj >= thr, adj, 0.0)
    return (adj > 0.5) | np.eye(N, dtype=bool)


def kernel(x: np.ndarray, params: dict) -> np.ndarray:
    import jax
    import jax.numpy as jnp

    x = np.asarray(x, dtype=np.float32)
    p = jax.tree.map(lambda a: np.asarray(a, dtype=np.float32), params)
    mask_np = _adjacency_mask(np.asarray(p['emb']))

    def _ln(h, g, b):
        m = h.mean(-1, keepdims=True)
        v = ((h - m) ** 2).mean(-1, keepdims=True)
        return (h - m) * jax.lax.rsqrt(v + EPS_LN) * g + b

    def _mha(q_in, kv, Wq, bq, Wk, bk, Wv, bv, Wo, bo):
        scale = np.float32(1.0 / np.sqrt(Wq.shape[-1]))
        q = jnp.einsum('bnd,dhc->bnhc', q_in, Wq) + bq
        k = jnp.einsum('bnd,dhc->bnhc', kv, Wk) + bk
        v = jnp.einsum('bnd,dhc->bnhc', kv, Wv) + bv
        sc = jnp.einsum('bqhc,bkhc->bhqk', q, k) * scale
        a = jax.nn.softmax(sc, axis=-1)
        o = jnp.einsum('bhqk,bkhc->bqhc', a, v)
        return jnp.einsum('bqhc,hco->bqo', o, Wo) + bo

    def _gat(h, mbias, gp):
        # Dense formulation with additive mask bias and no max-subtraction:
        # e is bounded (|e| < ~60) so exp cannot overflow in fp32, and masked
        # entries see exp(x - 1e9) == 0 exactly — identical numerics to the
        # reference's where(mask, e, -1e9) + softmax.
        W, a_s, a_d, bb = gp['W'], gp['a_src'], gp['a_dst'], gp['b']
        Bn, Nn, _ = h.shape
        H, C = a_s.shape
        hf = (h @ W).reshape(Bn, Nn, H, C)
        s = (hf * a_s).sum(-1)                       # [Bn, N, H]
        t = (hf * a_d).sum(-1)
        e = jax.nn.leaky_relu(
            s[:, :, None, :] + t[:, None, :, :], negative_slope=0.2)
        w = jnp.exp(e + mbias[None, :, :, None])     # [Bn, i, j, H]
        alpha = w / w.sum(axis=2, keepdims=True)
        out = jnp.einsum('bijh,bjhc->bihc', alpha, hf).reshape(Bn, Nn, H * C)
        return out + bb

    def fwd(xb, prm, mbias):
        # xb: [1, S, N, 3] — one batch element on this device
        Bq = xb.shape[0]
        xs = xb.reshape(Bq * S, N, 3)
        h = xs @ prm['proj_W'] + prm['proj_b']
        h = jax.nn.relu(_gat(h, mbias, prm['gat1']))
        h = _gat(h, mbias, prm['gat2'])
        spatial_out = h.reshape(Bq, S, N, 32)
        t_in = spatial_out.reshape(Bq * N, S, 32)
        attn = _mha(t_in, t_in, prm['tWq'], prm['tbq'], prm['tWk'], prm['tbk'],
                    prm['tWv'], prm['tbv'], prm['tWo'], prm['tbo'])
        out1 = _ln(t_in + attn, prm['ln1_g'], prm['ln1_b'])
        ffn = jax.nn.gelu(out1 @ prm['fW1'] + prm['fb1'],
                          approximate=False) @ prm['fW2'] + prm['fb2']
        t_out = _ln(out1 + ffn, prm['ln2_g'], prm['ln2_b'])
        temporal_out = t_out.reshape(Bq, N, S, 32)
        spatial_feats = spatial_out.mean(axis=1)
        temporal_feats = temporal_out.mean(axis=2)
        fused = _mha(spatial_feats, temporal_feats,
                     prm['cWq'], prm['cbq'], prm['cWk'], prm['cbk'],
                     prm['cWv'], prm['cbv'], prm['cWo'], prm['cbo'])
        hid = jax.nn.gelu(fused @ prm['rW1'] + prm['rb1'], approximate=False)
        return (hid @ prm['rW2'] + prm['rb2'])[..., 0]

    mbias_np = np.where(mask_np, np.float32(0.0), np.float32(-1e9))

    n_dev = min(8, jax.device_count())
    assert B % n_dev == 0
    xsh = x.reshape(n_dev, B // n_dev, S, N, 3)
    run = jax.pmap(fwd, in_axes=(0, None, None))
    out = run(xsh, p, jnp.asarray(mbias_np))
    return np.asarray(out).reshape(B, N).astype(np.float32)


if __name__ == '__main__':
    import reference
    ins = reference.setup_inputs()
    got = kernel(**{k: np.asarray(v) if not isinstance(v, dict) else v
                    for k, v in ins.items()})
    print(got.shape, got.dtype)
